# revision 1
# baseline (speedup 1.0000x reference)
"""Trainium2 Bass kernel for single-query gated cross-attention (DAttention).

Reference computation (per batch b, single query token at `pos`):
    q   = x[:, pos] @ Wq.T, scaled, split into 8 heads of 64
    kv  = context @ Wkv.T ; k, v = split(kv)
    dots = q @ k.T + attn_bias ; attn = softmax(mask(dots))
    out = (attn @ v) * sigmoid(x[:, pos] @ Wg.T + bg) @ Wo.T + bo

Key algebraic optimization: with a single query token the full K/V
projections (the dominant 69 GFLOP) are unnecessary:
    dots[b,h,j] = sum_c context[b,j,c] * qk[b,h,c],   qk = (q_scaled @ Wk_h)
    attn-weighted V = (sum_j attn[b,h,j] * context[b,j,c]) @ Wv_h.T
So the device only computes `dots` (context contraction with 16 folded
query vectors), the softmax, and the attention-weighted context sum
`acc[b,h,c]` — all memory-bound passes over context. The tiny O(batch)
pre/post folds (Wq/Wk fold, Wv fold, gating, output projection; ~90 MFLOP
total vs 69 GFLOP) run on host in fp32.

The attention bias (with masking as -1e30) is folded into the dots
matmul as an extra K=8 contraction block (lhsT = I8, rhs = bias rows),
so softmax is a single ACT exp per block with accum_out giving the
denominator. Softmax skips the max-subtraction: |dots + bias| <~ 10
here, far from fp32 exp overflow, and masked lanes underflow to exactly
0. The exp'd weights stay unnormalized; the denominator divides the
final [8, 512] accumulator.

Sharding: data-parallel over batch (16 batches / 8 cores = 2 per core).
No collectives needed; host gathers the [2, 8, 512] per-core results.
"""

import numpy as np
import ml_dtypes

import concourse.bass as bass
import concourse.bacc as bacc
import concourse.tile as tile
import concourse.mybir as mybir
from concourse.bass_utils import run_bass_kernel_spmd

BF16 = mybir.dt.bfloat16
F32 = mybir.dt.float32
NP_BF16 = ml_dtypes.bfloat16

N_CORES = 8
B = 16
N = 4096
DIM = 512
HEADS = 8
DIM_HEAD = 64
INNER = HEADS * DIM_HEAD
SCALE = DIM_HEAD ** -0.5
BPC = B // N_CORES          # batches per core (2)
KC = DIM // 128             # contraction chunks (4)
NJ = 8                      # dots j-blocks of 512
NT = N // 128               # token tiles of 128 (32)
NG = 4                      # natural-context tile groups of 8 token-tiles


def _build_nc():
    """Build + compile the SPMD single-core program (identical on all cores)."""
    nc = bacc.Bacc("TRN2", target_bir_lowering=False, debug=False,
                   num_devices=N_CORES)

    # DRAM I/O (per-core shapes)
    ctxT_d = nc.dram_tensor("ctxT", [BPC, KC, 128, N], BF16, kind="ExternalInput")
    ctxn_d = nc.dram_tensor("ctxn", [BPC, 128, NT, DIM], BF16, kind="ExternalInput")
    qkT_d = nc.dram_tensor("qkT", [KC, 128, BPC * HEADS], BF16, kind="ExternalInput")
    bias_d = nc.dram_tensor("biasT", [BPC, HEADS, N], F32, kind="ExternalInput")
    eye_d = nc.dram_tensor("eye8", [8, 8], BF16, kind="ExternalInput")
    acc_d = nc.dram_tensor("acc", [BPC, HEADS, DIM], F32, kind="ExternalOutput")

    with tile.TileContext(nc) as tc:
        with (
            tc.tile_pool(name="const", bufs=1) as const_pool,
            tc.tile_pool(name="ctxT", bufs=1) as ctxT_pool,
            tc.tile_pool(name="ctxn", bufs=1) as ctxn_pool,
            tc.tile_pool(name="attn", bufs=1) as attn_pool,
            tc.tile_pool(name="work", bufs=2) as work_pool,
            tc.tile_pool(name="pdots", bufs=4, space="PSUM") as pdots_pool,
            tc.tile_pool(name="ptr", bufs=2, space="PSUM") as ptr_pool,
            tc.tile_pool(name="pacc", bufs=1, space="PSUM") as pacc_pool,
        ):
            # ---- constants / small inputs (ACT HWDGE ring) ----
            qkT_sb = const_pool.tile([128, KC, BPC * HEADS], BF16, tag="qkT")
            nc.scalar.dma_start(out=qkT_sb[:], in_=qkT_d.rearrange("k p h -> p k h"))
            eye_sb = const_pool.tile([8, 8], BF16, tag="eye")
            nc.scalar.dma_start(out=eye_sb[:], in_=eye_d[:])
            bias_sb = []
            for b in range(BPC):
                t = const_pool.tile([HEADS, N], F32, tag=f"bias{b}",
                                    name=f"bias{b}")
                nc.scalar.dma_start(out=t[:], in_=bias_d[b])
                bias_sb.append(t)

            # ---- big context loads (SP HWDGE ring) ----
            # transposed context first (gates the dots matmuls)
            ctxT_sb = [[None] * KC for _ in range(BPC)]
            for b in range(BPC):
                for k in range(KC):
                    t = ctxT_pool.tile([128, N], BF16, tag=f"ctxT{b}{k}",
                                       name=f"ctxT{b}{k}")
                    nc.sync.dma_start(out=t[:], in_=ctxT_d[b, k])
                    ctxT_sb[b][k] = t
            # natural-layout context (consumed by the attn-weighted sum);
            # p-major DRAM layout -> one contiguous 8KB chunk per partition.
            ctxn_sb = [[None] * NG for _ in range(BPC)]
            for b in range(BPC):
                for g in range(NG):
                    t = ctxn_pool.tile([128, NT // NG, DIM], BF16,
                                       tag=f"ctxn{b}{g}", name=f"ctxn{b}{g}")
                    nc.sync.dma_start(out=t[:],
                                      in_=ctxn_d[b, :, bass.ts(g, NT // NG)])
                    ctxn_sb[b][g] = t

            # persistent SBUF intermediates (per batch, partitions 0-7)
            attnT = [attn_pool.tile([HEADS, N], BF16, tag=f"attnT{b}",
                                    name=f"attnT{b}") for b in range(BPC)]
            attn_nat = [attn_pool.tile([128, NT, HEADS], BF16,
                                       tag=f"attn_nat{b}", name=f"attn_nat{b}")
                        for b in range(BPC)]
            sums = attn_pool.tile([HEADS, BPC, NJ], F32, tag="sums")

            pacc = [pacc_pool.tile([HEADS, DIM], F32, tag=f"pa{b}", name=f"pa{b}")
                    for b in range(BPC)]

            # PE program order tracks data arrival: dots (ctxT, first half of
            # the DMA stream) for both batches, then transposes, then the
            # ctx_acc matmuls (ctxn, second half), then per-batch epilogue.
            for b in range(BPC):
                # dots + bias -> unnormalized attn weights (token-block major)
                for j in range(NJ):
                    pd = pdots_pool.tile([HEADS, 512], F32, tag="pd")
                    for k in range(KC):
                        nc.tensor.matmul(
                            pd[:],
                            lhsT=qkT_sb[:, k, bass.ts(b, HEADS)],
                            rhs=ctxT_sb[b][k][:, bass.ts(j, 512)],
                            start=(k == 0),
                            stop=(k == KC - 1),
                        )
                    # bias add in fp32 on the (otherwise idle) vector engine
                    nc.vector.tensor_tensor(
                        out=pd[:],
                        in0=pd[:],
                        in1=bias_sb[b][:, bass.ts(j, 512)],
                        op=mybir.AluOpType.add,
                    )
                    # exp -> bf16 weights + fp32 partial row-sum in one pass
                    nc.scalar.activation(
                        attnT[b][:, bass.ts(j, 512)], pd[:],
                        mybir.ActivationFunctionType.Exp,
                        accum_out=sums[:, b, j:j + 1],
                    )
            # transpose attn to token-major [128, NT, 8] per batch
            for b in range(BPC):
                for tq in range(NT // 4):
                    ptr = ptr_pool.tile([128, 4, HEADS], BF16, tag="ptr")
                    for tt in range(4):
                        jt = tq * 4 + tt
                        nc.tensor.transpose(
                            ptr[:, tt, :],
                            attnT[b][:, bass.ts(jt, 128)],
                            eye_sb[:],
                        )
                    nc.vector.tensor_copy(attn_nat[b][:, bass.ts(tq, 4)], ptr[:])
            stot = attn_pool.tile([HEADS, BPC], F32, tag="stot")
            rinv = attn_pool.tile([HEADS, BPC], F32, tag="rinv")
            for b in range(BPC):
                nc.vector.reduce_sum(stot[:, b:b + 1], sums[:, b],
                                     axis=mybir.AxisListType.X)
                nc.vector.reciprocal(rinv[:, b:b + 1], stot[:, b:b + 1])
            for b in range(BPC):
                # attention-weighted context sum (accumulate over token tiles)
                for jt in range(NT):
                    nc.tensor.matmul(
                        pacc[b][:],
                        lhsT=attn_nat[b][:, jt, :],
                        rhs=ctxn_sb[b][jt // (NT // NG)][:, jt % (NT // NG)],
                        start=(jt == 0),
                        stop=(jt == NT - 1),
                    )
                # normalize this batch and ship it while the next batch runs
                outt = work_pool.tile([HEADS, DIM], F32, tag="outt")
                nc.vector.tensor_scalar_mul(outt[:], pacc[b][:], rinv[:, b:b + 1])
                nc.scalar.dma_start(out=acc_d[b], in_=outt[:])

    nc.compile()
    return nc


_NC_CACHE = None


def _get_nc():
    global _NC_CACHE
    if _NC_CACHE is None:
        _NC_CACHE = _build_nc()
    return _NC_CACHE


def _host_prep(x, context, attn_bias, Wq, Wkv, Wg, bg, mask, context_mask, pos):
    """Fold the query-side projections and build per-core device inputs."""
    pos = int(pos)
    qx = np.asarray(x[:, pos, :], dtype=np.float32)              # [B, DIM]
    Wq = np.asarray(Wq, np.float32)
    Wkv = np.asarray(Wkv, np.float32)
    q = (qx @ Wq.T).reshape(B, HEADS, DIM_HEAD) * SCALE          # [B, 8, 64]
    Wk = Wkv[:INNER].reshape(HEADS, DIM_HEAD, DIM)               # [8, 64, DIM]
    qk = np.einsum("bhd,hdc->bhc", q, Wk)                        # [B, 8, DIM]

    # bias with masking folded in (-1e30 -> exp underflows to exactly 0)
    full_mask = (np.asarray(mask, bool).reshape(B, 1, 1)
                 & np.asarray(context_mask, bool).reshape(B, 1, N))
    biasT = np.where(full_mask,
                     np.asarray(attn_bias, np.float32).reshape(B, HEADS, N),
                     -1e30).astype(np.float32)

    ctx_bf = np.asarray(context, np.float32).astype(NP_BF16)     # [B, N, DIM]
    in_maps = []
    for c in range(N_CORES):
        bs = slice(c * BPC, (c + 1) * BPC)
        ctx_c = ctx_bf[bs]
        ctxT = np.ascontiguousarray(ctx_c.transpose(0, 2, 1)).reshape(
            BPC, KC, 128, N)
        ctxn = np.ascontiguousarray(
            ctx_c.reshape(BPC, NT, 128, DIM).transpose(0, 2, 1, 3))
        qkT = np.ascontiguousarray(
            qk[bs].transpose(2, 0, 1).reshape(DIM, BPC * HEADS)
        ).astype(NP_BF16).reshape(KC, 128, BPC * HEADS)
        in_maps.append({
            "ctxT": ctxT,
            "ctxn": ctxn,
            "qkT": qkT,
            "biasT": np.ascontiguousarray(biasT[bs]),
            "eye8": np.eye(8, dtype=NP_BF16),
        })
    return in_maps


def _host_epilogue(acc, x, Wkv, Wo, bo, Wg, bg, pos):
    """acc[b,h,c] -> out[b,1,dim] via the Wv fold, gating and Wo."""
    pos = int(pos)
    qx = np.asarray(x[:, pos, :], dtype=np.float32)
    Wv = np.asarray(Wkv, np.float32)[INNER:].reshape(HEADS, DIM_HEAD, DIM)
    out_v = np.einsum("bhc,hdc->bhd", acc, Wv).reshape(B, INNER)
    gates = qx @ np.asarray(Wg, np.float32).T + np.asarray(bg, np.float32)
    inner = out_v * (1.0 / (1.0 + np.exp(-gates)))
    out = inner @ np.asarray(Wo, np.float32).T + np.asarray(bo, np.float32)
    return out.reshape(B, 1, DIM).astype(np.float32)


def run_device(in_maps, trace=False):
    nc = _get_nc()
    return run_bass_kernel_spmd(nc, in_maps, list(range(N_CORES)), trace=trace)


def kernel(x, context, attn_bias, Wq, Wkv, Wo, bo, Wg, bg, mask, context_mask,
           pos, _trace=False, _results=None):
    in_maps = _host_prep(x, context, attn_bias, Wq, Wkv, Wg, bg,
                         mask, context_mask, pos)
    res = run_device(in_maps, trace=_trace)
    if _results is not None:
        _results.append(res)
    acc = np.concatenate([res.results[c]["acc"] for c in range(N_CORES)], axis=0)
    return _host_epilogue(acc.astype(np.float32), x, Wkv, Wo, bo, Wg, bg, pos)



# revision 4
# speedup vs baseline: 1.2211x; 1.2211x over previous
"""Trainium2 Bass kernel for single-query gated cross-attention (DAttention).

Reference computation (per batch b, single query token at `pos`):
    q   = x[:, pos] @ Wq.T, scaled, split into 8 heads of 64
    kv  = context @ Wkv.T ; k, v = split(kv)
    dots = q @ k.T + attn_bias ; attn = softmax(mask(dots))
    out = (attn @ v) * sigmoid(x[:, pos] @ Wg.T + bg) @ Wo.T + bo

Algebraic optimization: with a single query token the full K/V projections
(the dominant 69 GFLOP) are unnecessary:
    dots[b,h,j] = sum_c context[b,j,c] * qk[b,h,c],   qk = (q_scaled @ Wk_h)
    attn-weighted V = (sum_j attn[b,h,j] * context[b,j,c]) @ Wv_h.T
The device computes dots, exp, and the attention-weighted context sum;
the tiny O(batch) folds (Wq/Wk, Wv, gating, Wo, softmax normalize) run
on host.

Device design (v2, HBM-bound at ~12.7 MB/core vs 16.8 for v1):
  * dots pass reads a TRANSPOSED fp8e4m3 copy of context (4.2 MB/core)
    as the matmul stationary operand, producing dots TOKEN-MAJOR
    [128 tok, heads] directly -- no PE transposes of the attention row
    needed (v1 spent ~18us of PE + DVE on transposing attn to token
    major). qk is split into fp8 hi+lo pairs so only the context's own
    fp8 rounding perturbs dots (~1% weight error; well within 2e-2).
  * exp on ACT engine with scale=1/256 folding the fp8 scaling away;
    bias (pre-scaled x256, mask folded as -1e30) added on the otherwise
    idle DVE. attn stays bf16.
  * weighted-sum pass reads NATURAL bf16 context (8.4 MB/core): lhsT =
    attn tile [128,8], rhs = ctx tile [128,512] accumulated over the 32
    token tiles into PSUM [8,512]. fp8 here would breach tolerance (the
    rounding error of a weighted mean does not average down).
  * denominators via a second tiny matmul per tile (rhs = ones [128,1])
    reusing the same stationary; normalization happens on host.
  * DMA order: all ctxT (fp8) first -- dots fill the PE while the 2x
    larger ctxn stream follows; wsum trails the ctxn stream group by
    group, leaving only a ~2us tail after the last byte.

Sharding: data-parallel over batch (16 batches / 8 cores = 2 per core).
No collectives; host gathers per-core [2, 8, 512] sums + denominators.
"""

import numpy as np
import ml_dtypes

import concourse.bass as bass
import concourse.bacc as bacc
import concourse.tile as tile
import concourse.mybir as mybir
from concourse.bass_utils import run_bass_kernel_spmd

BF16 = mybir.dt.bfloat16
FP8 = mybir.dt.float8e4
F32 = mybir.dt.float32
NP_BF16 = ml_dtypes.bfloat16
# TRN2 supports the IEEE-ish E4M3 (max +-240), not e4m3fn (NCC_EVRF051)
NP_FP8 = ml_dtypes.float8_e4m3

N_CORES = 8
B = 16
N = 4096
DIM = 512
HEADS = 8
DIM_HEAD = 64
INNER = HEADS * DIM_HEAD
SCALE = DIM_HEAD ** -0.5
BPC = B // N_CORES          # batches per core (2)
KC = DIM // 128             # contraction chunks (4)
NT = N // 128               # token tiles of 128 (32)
GT = 8                      # token tiles per exp group
NG = NT // GT               # groups per batch (4)
QS = 256.0                  # fp8 scaling of qk (and bias)


def _build_nc():
    """Build + compile the SPMD single-core program (identical on all cores)."""
    nc = bacc.Bacc("TRN2", target_bir_lowering=False, debug=False,
                   num_devices=N_CORES)

    # DRAM I/O (per-core shapes)
    # ctxT: transposed context, fp8, token-halves for finer DMA pipelining
    ctxT_d = nc.dram_tensor("ctxT", [BPC, 2, KC, 128, N // 2], FP8,
                            kind="ExternalInput")
    # ctxn: natural context, bf16, token-quarters
    ctxn_d = nc.dram_tensor("ctxn", [BPC, 4, 128, NT // 4, DIM], BF16,
                            kind="ExternalInput")
    # qk2: folded query, fp8 hi/lo pairs, x256
    qk2_d = nc.dram_tensor("qk2", [KC, 128, BPC, 16], FP8,
                           kind="ExternalInput")
    # bias: token-major bias x256 with mask folded, bf16
    bias_d = nc.dram_tensor("biasT", [BPC, 128, NT, HEADS], BF16,
                            kind="ExternalInput")
    ones_d = nc.dram_tensor("ones1", [128, 1], BF16, kind="ExternalInput")
    acc_d = nc.dram_tensor("acc", [BPC, HEADS, DIM], F32,
                           kind="ExternalOutput")
    den_d = nc.dram_tensor("den", [BPC, HEADS, 1], F32,
                           kind="ExternalOutput")

    with tile.TileContext(nc) as tc:
        with (
            tc.tile_pool(name="const", bufs=1) as const_pool,
            tc.tile_pool(name="ctxT", bufs=1) as ctxT_pool,
            tc.tile_pool(name="ctxn", bufs=1) as ctxn_pool,
            tc.tile_pool(name="attn", bufs=1) as attn_pool,
            tc.tile_pool(name="work", bufs=2) as work_pool,
            tc.tile_pool(name="pd", bufs=2, space="PSUM") as pd_pool,
            tc.tile_pool(name="pacc", bufs=1, space="PSUM") as pacc_pool,
        ):
            # ---- big context loads first (SP HWDGE ring) ----
            # ctxT (fp8) leads: dots depend only on it; ctxn follows in
            # consumption order so wsum trails the stream tightly.
            ctxT_sb = [[None] * 2 for _ in range(BPC)]
            for b in range(BPC):
                for hf in range(2):
                    t = ctxT_pool.tile([128, KC, N // 2], FP8,
                                       tag=f"ctxT{b}{hf}", name=f"ctxT{b}{hf}")
                    nc.sync.dma_start(
                        out=t[:], in_=ctxT_d[b, hf].rearrange("k p n -> p k n"))
                    ctxT_sb[b][hf] = t
            ctxn_sb = [[None] * 4 for _ in range(BPC)]
            for b in range(BPC):
                for qt in range(4):
                    t = ctxn_pool.tile([128, NT // 4, DIM], BF16,
                                       tag=f"ctxn{b}{qt}", name=f"ctxn{b}{qt}")
                    nc.sync.dma_start(out=t[:], in_=ctxn_d[b, qt])
                    ctxn_sb[b][qt] = t

            # ---- small inputs (ACT HWDGE ring, concurrent with SP ring) ----
            qk2_sb = const_pool.tile([128, KC, BPC, 16], FP8, tag="qk2")
            nc.scalar.dma_start(out=qk2_sb[:],
                                in_=qk2_d.rearrange("k p b h -> p k b h"))
            ones_sb = const_pool.tile([128, 1], BF16, tag="ones")
            nc.scalar.dma_start(out=ones_sb[:], in_=ones_d[:])
            bias_sb = []
            for b in range(BPC):
                t = const_pool.tile([128, NT, HEADS], BF16, tag=f"bias{b}",
                                    name=f"bias{b}")
                nc.scalar.dma_start(out=t[:], in_=bias_d[b])
                bias_sb.append(t)

            # persistent attention weights, token-major [128, NT, 8] bf16
            attn = [attn_pool.tile([128, NT, HEADS], BF16, tag=f"attn{b}",
                                   name=f"attn{b}") for b in range(BPC)]

            pacc = [pacc_pool.tile([HEADS, DIM], F32, tag=f"pa{b}",
                                   name=f"pa{b}") for b in range(BPC)]
            pden = pacc_pool.tile([HEADS, BPC], F32, tag="pden")

            # ---- phase 1: dots + exp for both batches (needs only ctxT,
            # the first 4.2 MB of the DMA stream) ----
            for b in range(BPC):
                for g in range(NG):
                    pd = pd_pool.tile([128, GT * 16], F32, tag="pd")
                    for ti in range(GT):
                        jt = g * GT + ti            # global token tile
                        hf, loc = divmod(jt, NT // 2)
                        for k in range(KC):
                            nc.tensor.matmul(
                                pd[:, ti * 16:(ti + 1) * 16],
                                lhsT=ctxT_sb[b][hf][:, k,
                                                    loc * 128:(loc + 1) * 128],
                                rhs=qk2_sb[:, k, b, :],
                                start=(k == 0),
                                stop=(k == KC - 1),
                            )
                    # combine hi+lo qk halves and add bias (idle DVE).
                    # Each op reads PSUM at most once (NCC_IBVF027).
                    pdv = pd.rearrange("p (t two h) -> p t two h", two=2,
                                       h=HEADS)
                    dsum = work_pool.tile([128, GT, HEADS], F32, tag="dsum")
                    nc.vector.tensor_tensor(
                        out=dsum[:], in0=pdv[:, :, 0, :],
                        in1=bias_sb[b][:, g * GT:(g + 1) * GT, :],
                        op=mybir.AluOpType.add)
                    nc.vector.tensor_tensor(
                        out=dsum[:], in0=pdv[:, :, 1, :], in1=dsum[:],
                        op=mybir.AluOpType.add)
                    # exp (scale folds away the x256 fp8 scaling) -> bf16
                    nc.scalar.activation(
                        attn[b][:, g * GT:(g + 1) * GT, :], dsum[:],
                        mybir.ActivationFunctionType.Exp, scale=1.0 / QS)
            # ---- phase 2: attention-weighted context sum + denominators,
            # trailing the ctxn stream quarter by quarter ----
            for b in range(BPC):
                for jt in range(NT):
                    w = attn[b][:, jt, :]
                    nc.tensor.matmul(
                        pacc[b][:],
                        lhsT=w,
                        rhs=ctxn_sb[b][jt // (NT // 4)][:, jt % (NT // 4)],
                        start=(jt == 0),
                        stop=(jt == NT - 1),
                    )
                    nc.tensor.matmul(
                        pden[:, b:b + 1],
                        lhsT=w,
                        rhs=ones_sb[:],
                        start=(jt == 0),
                        stop=(jt == NT - 1),
                    )
                # ship unnormalized sums + denominator; host divides
                outt = work_pool.tile([HEADS, DIM], F32, tag="outt")
                nc.vector.tensor_copy(outt[:], pacc[b][:])
                nc.scalar.dma_start(out=acc_d[b], in_=outt[:])
                outd = work_pool.tile([HEADS, 1], F32, tag="outd")
                nc.vector.tensor_copy(outd[:], pden[:, b:b + 1])
                nc.scalar.dma_start(out=den_d[b], in_=outd[:])

    nc.compile()
    return nc


_NC_CACHE = None


def _get_nc():
    global _NC_CACHE
    if _NC_CACHE is None:
        _NC_CACHE = _build_nc()
    return _NC_CACHE


def _host_prep(x, context, attn_bias, Wq, Wkv, Wg, bg, mask, context_mask, pos):
    """Fold the query-side projections and build per-core device inputs."""
    pos = int(pos)
    qx = np.asarray(x[:, pos, :], dtype=np.float32)              # [B, DIM]
    Wq = np.asarray(Wq, np.float32)
    Wkv = np.asarray(Wkv, np.float32)
    q = (qx @ Wq.T).reshape(B, HEADS, DIM_HEAD) * SCALE          # [B, 8, 64]
    Wk = Wkv[:INNER].reshape(HEADS, DIM_HEAD, DIM)               # [8, 64, DIM]
    qk = np.einsum("bhd,hdc->bhc", q, Wk) * QS                   # [B, 8, DIM]

    # hi/lo fp8 split of qk: residual rounding error ~0.1%
    qk_hi = qk.astype(NP_FP8)
    qk_lo = (qk - qk_hi.astype(np.float32)).astype(NP_FP8)
    # [KC, 128, BPC_all=B, 16]
    qk2 = np.zeros((DIM, B, 16), dtype=NP_FP8)
    qk2[:, :, 0:8] = qk_hi.transpose(2, 0, 1)
    qk2[:, :, 8:16] = qk_lo.transpose(2, 0, 1)
    qk2 = qk2.reshape(KC, 128, B, 16)

    # bias x256 with masking folded in (-1e30 -> exp underflows to 0),
    # token-major [B, 128, NT, H]
    full_mask = (np.asarray(mask, bool).reshape(B, 1, 1)
                 & np.asarray(context_mask, bool).reshape(B, 1, N))
    biasf = np.where(full_mask,
                     np.asarray(attn_bias, np.float32).reshape(B, HEADS, N),
                     -1e30) * QS
    biasT = np.ascontiguousarray(
        biasf.reshape(B, HEADS, NT, 128).transpose(0, 3, 2, 1)).astype(NP_BF16)

    ctxf = np.asarray(context, np.float32)                       # [B, N, DIM]
    ones1 = np.ones((128, 1), dtype=NP_BF16)
    in_maps = []
    for c in range(N_CORES):
        bs = slice(c * BPC, (c + 1) * BPC)
        ctx_c = ctxf[bs]
        # [BPC, 2, KC, 128, N//2] fp8 transposed
        ctxT = np.ascontiguousarray(
            ctx_c.transpose(0, 2, 1).reshape(BPC, KC, 128, 2, N // 2)
            .transpose(0, 3, 1, 2, 4)).astype(NP_FP8)
        # [BPC, 4, 128, NT//4, DIM] bf16 natural
        ctxn = np.ascontiguousarray(
            ctx_c.reshape(BPC, 4, NT // 4, 128, DIM).transpose(0, 1, 3, 2, 4)
        ).astype(NP_BF16)
        in_maps.append({
            "ctxT": ctxT,
            "ctxn": ctxn,
            "qk2": np.ascontiguousarray(qk2[:, :, bs]),
            "biasT": np.ascontiguousarray(biasT[bs]),
            "ones1": ones1,
        })
    return in_maps


def _host_epilogue(acc, den, x, Wkv, Wo, bo, Wg, bg, pos):
    """acc[b,h,c]/den -> out[b,1,dim] via the Wv fold, gating and Wo."""
    pos = int(pos)
    qx = np.asarray(x[:, pos, :], dtype=np.float32)
    accn = acc / den.reshape(B, HEADS, 1)
    Wv = np.asarray(Wkv, np.float32)[INNER:].reshape(HEADS, DIM_HEAD, DIM)
    out_v = np.einsum("bhc,hdc->bhd", accn, Wv).reshape(B, INNER)
    gates = qx @ np.asarray(Wg, np.float32).T + np.asarray(bg, np.float32)
    inner = out_v * (1.0 / (1.0 + np.exp(-gates)))
    out = inner @ np.asarray(Wo, np.float32).T + np.asarray(bo, np.float32)
    return out.reshape(B, 1, DIM).astype(np.float32)


def run_device(in_maps, trace=False):
    nc = _get_nc()
    return run_bass_kernel_spmd(nc, in_maps, list(range(N_CORES)), trace=trace)


def kernel(x, context, attn_bias, Wq, Wkv, Wo, bo, Wg, bg, mask, context_mask,
           pos, _trace=False, _results=None):
    in_maps = _host_prep(x, context, attn_bias, Wq, Wkv, Wg, bg,
                         mask, context_mask, pos)
    res = run_device(in_maps, trace=_trace)
    if _results is not None:
        _results.append(res)
    acc = np.concatenate([res.results[c]["acc"] for c in range(N_CORES)],
                         axis=0).astype(np.float32)
    den = np.concatenate([res.results[c]["den"] for c in range(N_CORES)],
                         axis=0).astype(np.float32)
    return _host_epilogue(acc, den, x, Wkv, Wo, bo, Wg, bg, pos)
